# revision 1
# baseline (speedup 1.0000x reference)
"""Trainium2 Bass kernel for nn_DecoderBlock_85761906966851.

The reference decoder block's attention einsum ('bhss,bshd->bshd') takes the
DIAGONAL of the attention matrix, so token i only needs
    diag_prob_i[h] = exp(s_ii) / sum_{j<=i} exp(s_ij)
per head.  The kernel computes causal row-sums of exp(QK^T) (fused
exp+row-accumulate on the scalar engine), diagonal scores via an elementwise
q*k partition-block reduction, then a dense per-token pipeline
(Wo projection, LayerNorm, FFN, LayerNorm).

Sharding: 8 cores = 2 batches x 4 stride offsets; core (b, p) owns tokens
p::4 of batch b.  The stride-4 interleave equalizes causal work across cores
so one SPMD program fits all: row-slot a (128 rows) attends keys
[0, 512*(a+1)) with a per-core staircase mask on the last 512-key chunk
(keep col c iff c <= 4m + p), added in PSUM via an identity matmul.
No collectives; k is recomputed per core.

All matmul operands are float32r (fp32 with 11-bit mantissa, exact PE
product) pre-rounded on the host; DRAM layouts are pre-arranged host-side so
every DMA partition row is one contiguous run.
"""

import numpy as np

B, S, D, H, FF = 2, 2048, 512, 8, 2048
DK = D // H          # 64
P = 128
NT = 512             # tokens per core
NSLOT = 4
DO = D // P          # 4
KI = D // P          # 4
NFT = FF // P        # 16
EPS = 1e-3
NEG = -1.0e30

# packed f32 consts layout: [P, CF] = eps | bq(4) | bk(4) | b1(16) | keep(4) | 7x bcast(512)
CF_EPS, CF_BQ, CF_BK, CF_B1, CF_KEEP, CF_BC = 0, 1, 5, 9, 25, 29
BCN = ["bv", "bo", "b2", "g1", "be1", "g2", "be2"]
CF = CF_BC + 7 * D
# packed f32r consts: ident(128) | mask(512) | osel(32)
CR_ID, CR_MASK, CR_OSEL = 0, 128, 640
CR = 672

TRACE = False
LAST_EXEC_NS = None
_CACHE = {}


def to_f32r(a):
    """Round fp32 to fp32r (11-bit mantissa, round half up at bit 12)."""
    b = np.ascontiguousarray(a, dtype=np.float32).view(np.uint32)
    r = ((b.astype(np.uint64) + 0x800) & 0xFFFFF000).astype(np.uint32)
    return r.view(np.float32)


def _build_nc():
    import concourse.bass as bass
    import concourse.mybir as mybir
    import concourse.tile as tile
    from concourse import bacc

    f32 = mybir.dt.float32
    f32r = mybir.dt.float32r
    bf16 = mybir.dt.bfloat16
    Alu = mybir.AluOpType
    Act = mybir.ActivationFunctionType

    nc = bacc.Bacc(None, target_bir_lowering=False, debug=False)

    xTd = nc.dram_tensor("xT", [4, P, KI, 512], f32r, kind="ExternalInput")
    xTod = nc.dram_tensor("xTown", [P, KI, NT], f32r, kind="ExternalInput")
    Wqd = nc.dram_tensor("Wq", [P, KI, D], f32r, kind="ExternalInput")
    Wkd = nc.dram_tensor("Wk", [P, KI, D], f32r, kind="ExternalInput")
    Wvd = nc.dram_tensor("Wv", [P, KI, D], f32r, kind="ExternalInput")
    Wod = nc.dram_tensor("Wo", [P, KI, D], f32r, kind="ExternalInput")
    W1d = nc.dram_tensor("W1", [NFT, P, KI, P], f32r, kind="ExternalInput")
    W2d = nc.dram_tensor("W2", [NFT, P, D], f32r, kind="ExternalInput")
    cfd = nc.dram_tensor("cf", [P, CF], f32, kind="ExternalInput")
    crd = nc.dram_tensor("cr", [P, CR], f32r, kind="ExternalInput")
    outv = nc.dram_tensor("out", [NT, D], f32, kind="ExternalOutput")

    with tile.TileContext(nc) as tc:
        with (
            tc.tile_pool(name="const", bufs=1) as cst,
            tc.tile_pool(name="wgt", bufs=2) as wgt,
            tc.tile_pool(name="persist", bufs=1) as per,
            tc.tile_pool(name="stream", bufs=2) as stream,
            tc.tile_pool(name="xcs", bufs=2) as xcs,
            tc.tile_pool(name="wstr", bufs=3) as wstr,
            tc.tile_pool(name="expbuf", bufs=1) as expbuf,
        ):
            # ---------------- inputs resident in SBUF ----------------
            xTo = per.tile([P, KI, NT], f32r)
            wq_t = wgt.tile([P, KI, D], f32r, tag="w")
            wk_t = wgt.tile([P, KI, D], f32r, tag="w")
            for ki in range(KI):
                nc.sync.dma_start(out=xTo[:, ki, :], in_=xTod[:, ki, :])
                nc.sync.dma_start(out=wq_t[:, ki, :], in_=Wqd[:, ki, :])
            for ki in range(KI):
                nc.sync.dma_start(out=wk_t[:, ki, :], in_=Wkd[:, ki, :])
            cf = cst.tile([P, CF], f32)
            nc.sync.dma_start(out=cf, in_=cfd[:])
            cr = cst.tile([P, CR], f32r)
            nc.sync.dma_start(out=cr, in_=crd[:])

            eps_t = cf[:, CF_EPS:CF_EPS + 1]
            bq_t = cf[:, CF_BQ:CF_BQ + DO]
            bk_t = cf[:, CF_BK:CF_BK + DO]
            b1_t = cf[:, CF_B1:CF_B1 + NFT]
            keep_t = cf[:, CF_KEEP:CF_KEEP + NSLOT]
            bc = {n: cf[:, CF_BC + i * D:CF_BC + (i + 1) * D] for i, n in enumerate(BCN)}
            ident_r = cr[:, CR_ID:CR_ID + P]
            ident_f = ident_r.bitcast(f32)
            mask_t = cr[:, CR_MASK:CR_MASK + 512]
            osel_t = cr[:, CR_OSEL:CR_OSEL + 32].rearrange("p (o h) -> p o h", o=DO)

            qT = per.tile([P, DO, NT], f32r)
            kTo = per.tile([P, DO, NT], f32)
            kT = per.tile([P, DO, S], f32r)
            v_row = per.tile([P, NSLOT, D], f32, tag="v_xps")
            xbo = per.tile([P, NSLOT, D], f32r)
            xps = per.tile([P, NSLOT, D], f32r, tag="v_xps")
            xn1 = per.tile([P, NSLOT, D], f32)
            xnT = per.tile([P, KI, NT], f32r, tag="qkp_xnT")
            denom = per.tile([P, NSLOT, H], f32)
            d3b = per.tile([P, H], f32)
            rden = per.tile([P, NSLOT, H], f32)
            sii_eT = per.tile([H, NT], f32)
            dp = per.tile([P, NSLOT, H], f32)
            qkp = per.tile([P, DO, NT], f32r, tag="qkp_xnT")
            out_sb = per.tile([P, NSLOT, D], f32)

            def ln(src, dst, gname, bname):
                st = stream.tile([P, 6], f32, tag="ln_st", name="ln_st")
                nc.vector.bn_stats(out=st, in_=src)
                mv = stream.tile([P, 2], f32, tag="ln_mv", name="ln_mv")
                nc.vector.bn_aggr(out=mv, in_=st)
                nc.scalar.activation(out=mv[:, 1:2], in_=mv[:, 1:2],
                                     func=Act.Sqrt, bias=eps_t)
                nc.vector.reciprocal(out=mv[:, 1:2], in_=mv[:, 1:2])
                nm = stream.tile([P, 1], f32, tag="ln_nm", name="ln_nm")
                nc.vector.tensor_scalar(out=nm, in0=mv[:, 0:1], scalar1=mv[:, 1:2],
                                        scalar2=-1.0, op0=Alu.mult, op1=Alu.mult)
                nc.scalar.activation(out=dst, in_=src, func=Act.Identity,
                                     bias=nm, scale=mv[:, 1:2])
                nc.vector.tensor_tensor(dst, dst, bc[gname], Alu.mult)
                nc.gpsimd.tensor_tensor(dst, dst, bc[bname], Alu.add)

            # ===== fused phase 1+2: projections, kT, causal exp row-sums =====
            # kT chunks and other PE work interleave with the ACT-bound exp
            # stream (keeps the PE dense and the HAM clock warm).  Sequential
            # PSUM scopes A-D; each carries a "fil" tag for non-score matmuls.
            wr = [None] * NSLOT

            def kT_chunk(pool, ck, xc):
                for do in range(DO):
                    ps = pool.tile([P, 512], f32, tag="fil", name="pp_k", bufs=2)
                    for ki in range(KI):
                        nc.tensor.matmul(
                            ps, wk_t[:, ki, do * P:(do + 1) * P], xc[:, ki, :],
                            start=(ki == 0), stop=(ki == KI - 1))
                    nc.vector.tensor_scalar_add(
                        kT[:, do, ck * 512:(ck + 1) * 512], ps, bk_t[:, do:do + 1])

            def score_mms(pool, a, h, tag, kw, nb):
                po, pr = (h % 2) * DK, h // 2
                ps = pool.tile([P, kw], f32, tag=tag, name=tag, bufs=nb)
                for ck in range(a + 1):
                    nc.tensor.matmul(
                        ps[:, ck * 512:(ck + 1) * 512],
                        qT[po:po + DK, pr, a * P:(a + 1) * P],
                        kT[po:po + DK, pr, ck * 512:(ck + 1) * 512],
                        start=True, stop=True)
                nc.vector.tensor_tensor(ps[:, a * 512:(a + 1) * 512],
                                        ps[:, a * 512:(a + 1) * 512],
                                        mask_t.bitcast(f32), Alu.add)
                esc = expbuf.tile([P, 1536], bf16, tag="esc", name="esc")
                nc.scalar.activation(esc[:, :kw], ps, Act.Exp,
                                     accum_out=denom[:, a, h:h + 1])

            def dp_only(a, pool):
                nc.vector.reciprocal(rden[:, a, :], denom[:, a, :])
                ps = pool.tile([P, H], f32, tag="fil", name="sT", bufs=2)
                nc.tensor.matmul(ps, sii_eT[:, a * P:(a + 1) * P],
                                 ident_f[:H, :H],
                                 is_transpose=True, start=True, stop=True)
                nc.vector.tensor_tensor(dp[:, a, :], ps, rden[:, a, :], Alu.mult)
                nc.vector.tensor_scalar_mul(dp[:, a, :], dp[:, a, :],
                                            keep_t[:, a:a + 1])

            # ---- scope A: qT, kT0, scores slot 0, kTo, s_ii ----
            with tc.tile_pool(name="scA", bufs=1, space="PSUM") as sA:
                xc0 = xcs.tile([P, KI, 512], f32r, tag="xc", name="xc0")
                nc.sync.dma_start(out=xc0, in_=xTd[0])
                xc1 = xcs.tile([P, KI, 512], f32r, tag="xc", name="xc1")
                nc.sync.dma_start(out=xc1, in_=xTd[1])
                for do in range(DO):
                    ps = sA.tile([P, NT], f32, tag="fil", name="pp_q", bufs=2)
                    for ki in range(KI):
                        nc.tensor.matmul(
                            ps, wq_t[:, ki, do * P:(do + 1) * P], xTo[:, ki, :],
                            start=(ki == 0), stop=(ki == KI - 1))
                    nc.vector.tensor_scalar_add(qT[:, do, :], ps,
                                                bq_t[:, do:do + 1])
                wv_t = wgt.tile([P, KI, D], f32r, tag="w")
                nc.sync.dma_start(out=wv_t, in_=Wvd[:])
                kT_chunk(sA, 0, xc0)
                for h in range(4):
                    score_mms(sA, 0, h, "sc0", 512, 4)
                for do in range(DO):
                    ps = sA.tile([P, NT], f32, tag="fil", name="pp_ko", bufs=2)
                    for ki in range(KI):
                        nc.tensor.matmul(
                            ps, wk_t[:, ki, do * P:(do + 1) * P], xTo[:, ki, :],
                            start=(ki == 0), stop=(ki == KI - 1))
                    nc.vector.tensor_scalar_add(kTo[:, do, :], ps,
                                                bk_t[:, do:do + 1])
                for h in range(4, H):
                    score_mms(sA, 0, h, "sc0", 512, 4)
                nc.vector.tensor_tensor(qkp[:], qT[:].bitcast(f32), kTo[:], Alu.mult)
                ps = sA.tile([H, NT], f32, tag="fil", name="fx_sii", bufs=2)
                for dt in range(DO):
                    nc.tensor.matmul(ps, osel_t[:, dt, :], qkp[:, dt, :],
                                     start=(dt == 0), stop=(dt == DO - 1))
                nc.scalar.activation(sii_eT, ps, Act.Exp)
                wo_t = wgt.tile([P, KI, D], f32r, tag="w")
                nc.sync.dma_start(out=wo_t, in_=Wod[:])
                dp_only(0, sA)

            # ---- scope B: kT1, scores slot 1 ----
            with tc.tile_pool(name="scB", bufs=1, space="PSUM") as sB:
                xc2 = xcs.tile([P, KI, 512], f32r, tag="xc", name="xc2")
                nc.sync.dma_start(out=xc2, in_=xTd[2])
                kT_chunk(sB, 1, xc1)
                for h in range(H):
                    score_mms(sB, 1, h, "sc1", 1024, 3)
                dp_only(1, sB)

            # ---- scope C: kT2, scores slot 2, v rows ----
            with tc.tile_pool(name="scC", bufs=1, space="PSUM") as sC:
                xc3 = xcs.tile([P, KI, 512], f32r, tag="xc", name="xc3")
                nc.sync.dma_start(out=xc3, in_=xTd[3])
                kT_chunk(sC, 2, xc2)
                for h in range(4):
                    score_mms(sC, 2, h, "sc2", 1536, 2)
                for s in range(2):
                    ps = sC.tile([P, D], f32, tag="fil", name="fx_v", bufs=2)
                    for ki in range(KI):
                        nc.tensor.matmul(
                            ps, xTo[:, ki, s * P:(s + 1) * P], wv_t[:, ki, :],
                            start=(ki == 0), stop=(ki == KI - 1))
                    nc.vector.tensor_tensor(v_row[:, s, :], ps, bc["bv"], Alu.add)
                for h in range(4, H):
                    score_mms(sC, 2, h, "sc2", 1536, 2)
                for s in range(2, NSLOT):
                    ps = sC.tile([P, D], f32, tag="fil", name="fx_v", bufs=2)
                    for ki in range(KI):
                        nc.tensor.matmul(
                            ps, xTo[:, ki, s * P:(s + 1) * P], wv_t[:, ki, :],
                            start=(ki == 0), stop=(ki == KI - 1))
                    nc.vector.tensor_tensor(v_row[:, s, :], ps, bc["bv"], Alu.add)
                dp_only(2, sC)

            # ---- scope D: kT3, scores slot 3, x rows ----
            with (
                tc.tile_pool(name="scD", bufs=1, space="PSUM") as sD,
                tc.tile_pool(name="scD3", bufs=2, space="PSUM") as sD3,
            ):
                kT_chunk(sD, 3, xc3)
                for h in range(H):
                    po, pr = (h % 2) * DK, h // 2
                    pa = sD.tile([P, 1024], f32, tag="sc3a", name="sc3a", bufs=1)
                    pb = sD3.tile([P, 1024], f32, tag="sc3b", name="sc3b")
                    for ck in range(4):
                        tgt = pa if ck < 2 else pb
                        off = (ck % 2) * 512
                        nc.tensor.matmul(
                            tgt[:, off:off + 512],
                            qT[po:po + DK, pr, 3 * P:4 * P],
                            kT[po:po + DK, pr, ck * 512:(ck + 1) * 512],
                            start=True, stop=True)
                    nc.vector.tensor_tensor(pb[:, 512:1024], pb[:, 512:1024],
                                            mask_t.bitcast(f32), Alu.add)
                    esa = expbuf.tile([P, 1024], bf16, tag="esa", name="esa")
                    nc.scalar.activation(esa, pa, Act.Exp,
                                         accum_out=denom[:, 3, h:h + 1])
                    esb = expbuf.tile([P, 1024], bf16, tag="esb", name="esb")
                    nc.scalar.activation(esb, pb, Act.Exp,
                                         accum_out=d3b[:, h:h + 1])
                    if h == 2:  # x rows as PE filler mid-slot3
                        for s in range(NSLOT):
                            psr = sD.tile([P, D], f32r, tag="fil", name="fx_x", bufs=2)
                            for ki in range(KI):
                                nc.tensor.transpose(
                                    psr[:, ki * P:(ki + 1) * P],
                                    xTo[:, ki, s * P:(s + 1) * P], ident_r)
                            nc.vector.tensor_tensor(xbo[:, s, :],
                                                    psr.bitcast(f32),
                                                    bc["bo"], Alu.add)
                nc.vector.tensor_tensor(denom[:, 3, :], denom[:, 3, :],
                                        d3b, Alu.add)

            # ============ phase 3: attn out + LN1 (from PSUM) ============
            with tc.tile_pool(name="pe", bufs=2, space="PSUM") as pe:
                dp_only(3, pe)
                for a in range(NSLOT):
                    w = stream.tile([P, D], f32, tag=f"wr{a}", name=f"wr{a}")
                    nc.vector.tensor_tensor(
                        w.rearrange("p (h d) -> p h d", h=H),
                        v_row[:, a, :].rearrange("p (h d) -> p h d", h=H),
                        dp[:, a, :, None].to_broadcast([P, H, DK]), Alu.mult)
                    wr[a] = w
                    pw = pe.tile([P, KI, P], f32, tag="pw", name="pw")
                    for ki in range(KI):
                        nc.tensor.transpose(pw[:, ki, :],
                                            wr[a][:, ki * P:(ki + 1) * P], ident_f)
                    wTs = stream.tile([P, KI, P], f32r, tag="wTs", name="wTs")
                    nc.vector.tensor_copy(wTs, pw)
                    ps = pe.tile([P, D], f32, tag="po", name="po")
                    for ki in range(KI):
                        nc.tensor.matmul(ps, wTs[:, ki, :], wo_t[:, ki, :],
                                         start=(ki == 0), stop=False)
                    nc.tensor.matmul(ps, ident_r, xbo[:, a, :],
                                     start=False, stop=True)
                    ln(ps, xn1[:, a, :], "g1", "be1")

                for a in range(NSLOT):
                    pt = pe.tile([P, KI, P], f32, tag="pw", name="pt")
                    for ki in range(KI):
                        nc.tensor.transpose(pt[:, ki, :],
                                            xn1[:, a, ki * P:(ki + 1) * P], ident_f)
                    for ki in range(KI):
                        nc.vector.tensor_copy(xnT[:, ki, a * P:(a + 1) * P],
                                              pt[:, ki, :])

            # ============ phase 4: FFN, LN2, store ============
            with (
                tc.tile_pool(name="ph", bufs=2, space="PSUM") as ph,
                tc.tile_pool(name="py", bufs=1, space="PSUM") as py,
            ):
                psy = [py.tile([P, D], f32, tag=f"y{a}", name=f"y{a}")
                       for a in range(NSLOT)]
                for ft in range(NFT):
                    w1c = wstr.tile([P, KI, P], f32r, tag="w1c", name="w1c")
                    nc.sync.dma_start(out=w1c, in_=W1d[ft])
                    w2c = wstr.tile([P, D], f32r, tag="w2c", name="w2c")
                    nc.sync.dma_start(out=w2c, in_=W2d[ft])
                    psh = ph.tile([P, NT], f32, tag="h", name="psh")
                    for ki in range(KI):
                        nc.tensor.matmul(psh, w1c[:, ki, :], xnT[:, ki, :],
                                         start=(ki == 0), stop=(ki == KI - 1))
                    hr = stream.tile([P, NT], f32r, tag="hr", name="hr")
                    nc.vector.tensor_scalar(out=hr, in0=psh,
                                            scalar1=b1_t[:, ft:ft + 1], scalar2=0.0,
                                            op0=Alu.add, op1=Alu.max)
                    for a in range(NSLOT):
                        nc.tensor.matmul(psy[a], hr[:, a * P:(a + 1) * P], w2c,
                                         start=(ft == 0), stop=False)
                    if ft == 0:
                        # r2 residual (xn1 + b2) folded into the accumulation;
                        # DVE is idle here
                        for a in range(NSLOT):
                            nc.vector.tensor_tensor(xps[:, a, :], xn1[:, a, :],
                                                    bc["b2"], Alu.add)
                for a in range(NSLOT):
                    nc.tensor.matmul(psy[a], ident_r, xps[:, a, :],
                                     start=False, stop=True)
                out_re = outv[:].rearrange("(a p) d -> p a d", p=P)
                for a in range(NSLOT):
                    ln(psy[a], out_sb[:, a, :], "g2", "be2")
                    nc.sync.dma_start(out=out_re[:, a, :], in_=out_sb[:, a, :])

    nc.compile()
    return nc


def _get_nc():
    if "nc" not in _CACHE:
        _CACHE["nc"] = _build_nc()
    return _CACHE["nc"]


def _rearr_w(w):
    # [Din, N] -> [P, KI, N] with [p, o, n] = w[o*128+p, n]
    return np.ascontiguousarray(
        to_f32r(w).reshape(KI, P, -1).transpose(1, 0, 2))


def kernel(x, lengths, Wq, bq, Wk, bk, Wv, bv, Wo, bo, W1, b1, W2, b2,
           gamma1, beta1, gamma2, beta2):
    global LAST_EXEC_NS
    from concourse.bass_utils import run_bass_kernel_spmd

    x = np.asarray(x, dtype=np.float32)
    lengths = np.asarray(lengths, dtype=np.int32)
    f32a = lambda a: np.asarray(a, dtype=np.float32)

    pad = (np.arange(S)[None, :] < lengths[:, None]).astype(np.float32)
    xm = x * pad[:, :, None]

    # W1 [D, FF] -> [NFT, P, KI, P]; W2 [FF, D] -> [NFT, P, D]
    w1p = np.ascontiguousarray(
        to_f32r(f32a(W1)).reshape(KI, P, NFT, P).transpose(2, 1, 0, 3))
    w2p = np.ascontiguousarray(to_f32r(f32a(W2)).reshape(NFT, P, D))

    # packed consts
    cfv = np.zeros((P, CF), dtype=np.float32)
    cfv[:, CF_EPS] = EPS
    cfv[:, CF_BQ:CF_BQ + DO] = f32a(bq).reshape(DO, P).T
    cfv[:, CF_BK:CF_BK + DO] = f32a(bk).reshape(DO, P).T
    cfv[:, CF_B1:CF_B1 + NFT] = f32a(b1).reshape(NFT, P).T
    for i, v in enumerate([bv, bo, b2, gamma1, beta1, gamma2, beta2]):
        cfv[:, CF_BC + i * D:CF_BC + (i + 1) * D] = f32a(v)[None, :]

    osel = np.zeros((P, DO, H), dtype=np.float32)
    for dt in range(DO):
        osel[:DK, dt, 2 * dt] = 1.0
        osel[DK:, dt, 2 * dt + 1] = 1.0

    common = dict(Wq=_rearr_w(f32a(Wq)), Wk=_rearr_w(f32a(Wk)),
                  Wv=_rearr_w(f32a(Wv)), Wo=_rearr_w(f32a(Wo)),
                  W1=w1p, W2=w2p)

    cols = np.arange(512)[None, :]
    rows = np.arange(P)[:, None]

    in_maps = []
    for c in range(8):
        b, p = c // 4, c % 4
        xTb = to_f32r(np.ascontiguousarray(xm[b].T))        # [D, S]
        # [4, P, KI, 512]: [ck, p, o, s] = xT[o*128+p, ck*512+s]
        xt4 = np.ascontiguousarray(
            xTb.reshape(KI, P, 4, 512).transpose(2, 1, 0, 3))
        xto = np.ascontiguousarray(
            xTb[:, p::4].reshape(KI, P, NT).transpose(1, 0, 2))
        m = to_f32r(np.where(cols <= 4 * rows + p, 0.0, NEG).astype(np.float32))
        tloc = p + 4 * (np.arange(NSLOT)[None, :] * P + rows)
        keep = (tloc < lengths[b]).astype(np.float32)
        cfc = cfv.copy()
        cfc[:, CF_KEEP:CF_KEEP + NSLOT] = keep
        crc = np.zeros((P, CR), dtype=np.float32)
        crc[:, CR_ID:CR_ID + P] = np.eye(P, dtype=np.float32)
        crc[:, CR_MASK:CR_MASK + 512] = m
        crc[:, CR_OSEL:CR_OSEL + 32] = osel.reshape(P, 32)
        in_maps.append(dict(xT=xt4, xTown=xto, cf=cfc, cr=crc, **common))

    nc = _get_nc()
    res = run_bass_kernel_spmd(nc, in_maps, list(range(8)), trace=TRACE)
    LAST_EXEC_NS = res.exec_time_ns

    out = np.empty((B, S, D), dtype=np.float32)
    for c in range(8):
        b, p = c // 4, c % 4
        out[b, p::4, :] = res.results[c]["out"]
    return out

